# revision 23
# baseline (speedup 1.0000x reference)
"""Trainium2 Bass kernel for nn_ExplainerCompatibleGinGru.

Math: the reference pads the batch with 31 zero graphs, splits the node dim
into two 36-node graphs (ad = rows 0:36, dis = rows 36:72), runs 3 GIN layers
with sum-pooling, packs [ad x (L-1), dis] as a GRU sequence per batch
element, and returns out[0] -- which depends ONLY on graph 0 (ad), graph 32
(dis) and L = LOS_batch[0].  So the kernel computes: GIN on the stacked
72-node 2-graph block, an L-step GRU on one sequence, and a tiny classifier.

Runs replicated on all 8 cores (cross-core collectives measured ~75us in
this environment -- useless; per-core HBM bandwidth is independent, so
sharding weights would need a collective and loses).  The run is bound by
~8MB of fp16 weight DMA at ~330GB/s/core; everything else hides under it.

Layout tricks:
- aggregation is a dense matmul with the (A+I)^T block-diagonal operator
- LN: bn_stats halves, rstd via one Rsqrt ACT; apply is a fused 2-scalar DVE
  op + ACT-identity half; PE transpose then relu(g*x+be) is ONE ACT op per
  chunk (g/be are per-partition scalars in the transposed layout)
- gi (Wih matvec) flushes per kc-chunk as wiht DMA quarters land
- GRU per-step biases enter PSUM via one [8,128]^T @ eye8 closer (r+z gates
  share a psum tile so ONE sigmoid covers both)
- GRU state h is split into two [128,2] fp16 tiles so the next step's first
  contraction chunks start before the second half is written
- step 0 needs no matmuls (h0 = 0); gibT setup overlaps its ACT/DVE chain
- DMA: 3 queues (sync/scalar/gpsimd) with strict priority order; wc1 last
"""

import os
import numpy as np
import ml_dtypes  # noqa: F401

F16 = np.float16

H = 512
LN_EPS = 1e-5

_prog_cache = {}
last_run_info = {}


def _pack_kchunks(w, ncols):
    """[K, N] weight -> [128, (K//128)*N], chunk kc at cols [N*kc, N*(kc+1))."""
    k, n = w.shape
    assert k % 128 == 0 and n == ncols
    nk = k // 128
    return np.ascontiguousarray(
        w.reshape(nk, 128, n).transpose(1, 0, 2).reshape(128, nk * n))


def _prep_inputs(inputs):
    f32 = np.float32

    def bf(x):
        return np.asarray(x, f32).astype(F16)

    x = np.asarray(inputs['x_embedded'], f32)
    tei = np.asarray(inputs['template_edge_index']).astype(np.int64)
    L = int(np.asarray(inputs['LOS_batch']).reshape(-1)[0])

    A = np.zeros((36, 36), f32)
    np.add.at(A, (tei[1], tei[0]), 1.0)
    Mp = A + np.eye(36, dtype=f32)
    m72 = np.zeros((72, 72), f32)
    m72[:36, :36] = Mp.T
    m72[36:, 36:] = Mp.T

    W = {k: np.asarray(v, f32) for k, v in inputs.items()
         if k not in ('x_embedded', 'template_edge_index', 'LOS_batch')}

    # axe [72,688]: x0 | eye72 | m72 | w1a (rows 0:32)
    axe = np.zeros((72, 688), F16)
    axe[:, 0:32] = bf(x)
    axe[:, 32:104] = bf(np.eye(72, dtype=f32))
    axe[:, 104:176] = bf(m72)
    axe[0:32, 176:688] = bf(W['W1a'])
    # a3 [1, 2122]: brows(2050) | ones72
    a3 = np.zeros((1, 2122), F16)
    a3[0, 0:2050] = bf(np.concatenate(
        [W['b1a'], W['b1b'], W['bha'], W['bhb'], [0.0], [0.0]]))
    a3[0, 2050:2122] = 1.0
    # a4 [128,136]: eye128 | wc2(8 cols)
    a4 = np.zeros((128, 136), F16)
    a4[:, 0:128] = np.eye(128, dtype=F16)
    a4[:, 128:136] = bf(np.ascontiguousarray(W['Wc2'].reshape(8, 128).T))
    # a5 [8,264]: bc1t(8r x 128) | bhhnt(4r x 128) | eye8
    a5 = np.zeros((8, 264), F16)
    a5[0:8, 0:128] = bf(W['bc1'].reshape(8, 128))
    a5[0:4, 128:256] = bf(W['bhh'][2 * H:].reshape(4, 128))
    a5[0:8, 256:264] = np.eye(8, dtype=F16)

    # f32v layout:
    #  0:24  combo24: [p, 2j+g] = bih[p+128j] (+ bhh[p+128j] for j<8)
    # 24:28  bhh_n tile; 28 bc2; 29:33 g1T; 33:37 be1T; 37:41 ghT; 41:45 behT
    # 45:49  b1bT; 49:53 bhbT
    f32v = np.zeros((128, 53), f32)
    bih_t = W['bih'].reshape(12, 128).T
    bhh_t = W['bhh'].reshape(12, 128).T
    combo = bih_t.copy()
    combo[:, 0:8] += bhh_t[:, 0:8]
    f32v[:, 0:24:2] = combo
    f32v[:, 1:24:2] = combo
    f32v[:, 24:28] = bhh_t[:, 8:12]
    f32v[:, 28] = W['bc2'][0]
    f32v[:, 29:33] = W['g1'].reshape(4, 128).T
    f32v[:, 33:37] = W['be1'].reshape(4, 128).T
    f32v[:, 37:41] = W['gh'].reshape(4, 128).T
    f32v[:, 41:45] = W['beh'].reshape(4, 128).T
    f32v[:, 45:49] = W['b1b'].reshape(4, 128).T
    f32v[:, 49:53] = W['bhb'].reshape(4, 128).T

    gw16 = np.concatenate([
        _pack_kchunks(W['W1b'], H), _pack_kchunks(W['Wha'], H),
        _pack_kchunks(W['Whb'], H)], axis=1).astype(F16)

    blobs = {
        'axe': axe, 'a3': a3, 'a4': a4, 'a5': a5,
        'f32v': f32v,
        'gw16': gw16,
        'wiht': bf(_pack_kchunks(np.ascontiguousarray(W['Wih'].T), 1536)),
        'whht': bf(_pack_kchunks(np.ascontiguousarray(W['Whh'].T), 1536)),
        'wc1': bf(_pack_kchunks(W['Wc1'], 1024)),
    }
    return blobs, L


def _emit(ctx, tc, d, out_dram, L):
    import concourse.mybir as mybir
    nc = tc.nc
    f32 = mybir.dt.float32
    f16 = mybir.dt.float16
    AF = mybir.ActivationFunctionType
    AL = mybir.AluOpType

    wts = ctx.enter_context(tc.tile_pool(name="wts", bufs=1))
    act = ctx.enter_context(tc.tile_pool(name="act", bufs=1))
    tmp = ctx.enter_context(tc.tile_pool(name="tmp", bufs=2))
    pbig = ctx.enter_context(tc.tile_pool(name="pbig", bufs=2, space="PSUM"))
    psm = ctx.enter_context(tc.tile_pool(name="psm", bufs=3, space="PSUM"))
    pgi = ctx.enter_context(tc.tile_pool(name="pgi", bufs=1, space="PSUM"))

    # ---- DMA: queue plan ----
    # The 16 DMA engines drain descriptors in global enqueue order across
    # queues, so enqueue order == need order matters more than queue choice.
    # scalar (HWDGE): sqrt-table prefetch first, then the tiny GIN operands.
    # sync (SP, HWDGE, no compute): the big weight stream in need order.
    # gpsimd (SWDGE, slow start): late small blobs only.

    # tiny GIN operands first on the scalar queue (before the table
    # prefetch, which would delay their descriptor enqueue by ~1.6us)
    axe = wts.tile([72, 688], f16, tag='axe')
    nc.scalar.dma_start(axe[:, :], d['axe'])
    a3 = wts.tile([1, 2122], f16, tag='a3')
    nc.scalar.dma_start(a3[:, :], d['a3'])
    f32v = wts.tile([128, 53], f32, tag='f32v')
    nc.scalar.dma_start(f32v[:, :], d['f32v'])

    # prefetch the Sqrt ACT table (first LN would otherwise stall ~2.7us)
    sc1 = act.tile([1, 1], f32, tag='sc1')
    nc.vector.memset(sc1[:, :], 1.0)
    sc2 = act.tile([1, 1], f32, tag='sc2')
    eps = act.tile([72, 1], f32, tag='eps')
    nc.vector.memset(eps[:, :], LN_EPS)
    nc.scalar.activation(sc2[:, :], sc1[:, :], AF.Sqrt)

    x0s = axe[:, 0:32]
    eye72 = axe[:, 32:104]
    m72 = axe[:, 104:176]
    w1a = axe[0:32, 176:688]
    ones72 = a3[0:1, 2050:2122]

    # sync: big weights in strict need order
    gw16 = wts.tile([128, 3 * 4 * H], f16, tag='gw16')
    wiht_t = [wts.tile([128, 3 * 1536], f16, tag=f'wiht{q}',
                       name=f'wiht{q}') for q in range(4)]
    nc.sync.dma_start(gw16[:, 0:2048], d['gw16'][:, 0:2048])        # w1b
    nc.sync.dma_start(gw16[:, 2048:4096], d['gw16'][:, 2048:4096])  # wha
    nc.sync.dma_start(gw16[:, 4096:6144], d['gw16'][:, 4096:6144])  # whb
    for q in range(4):
        nc.sync.dma_start(wiht_t[q][:, :], d['wiht'][:, 4608 * q:4608 * (q + 1)])
    whht = wts.tile([128, 4 * 1536], f16, tag='whht')
    nc.sync.dma_start(whht[:, :], d['whht'])
    wc1 = wts.tile([128, 4 * 1024], f16, tag='wc1')
    nc.sync.dma_start(wc1[:, :], d['wc1'])

    # gpsimd: small blobs needed only by the GRU/classifier phases
    a5 = wts.tile([8, 264], f16, tag='a5')
    nc.gpsimd.dma_start(a5[:, :], d['a5'])
    a4 = wts.tile([128, 136], f16, tag='a4')
    nc.gpsimd.dma_start(a4[:, :], d['a4'])

    eye128 = a4[:, 0:128]
    wc2 = a4[:, 128:136]
    bc1t = a5[0:8, 0:128]
    bhhnt = a5[0:4, 128:256]
    eye8 = a5[0:8, 256:264]
    eye4 = a5[0:4, 256:260]

    def wiht_chunk(kc, j):
        q, r = divmod(kc, 3)
        base = 1536 * r + 128 * j
        return wiht_t[q][:, base:base + 128]

    featsT = act.tile([128, 24], f16, tag='featsT')
    gi_ps = pgi.tile([128, 24], f32, tag='gi')

    # PE clock-warming filler: the tensor engine ramps its clock only under
    # sustained load; the GIN phase has ~2.5us PE-idle LN gaps per layer
    # that keep it at ~1GHz.  Junk transposes during the gaps keep it hot.
    _fill_n = [0]

    def pe_filler(n):
        junk_ps = psm.tile([128, 72], f16, tag='psm')
        for i in range(n):
            nc.tensor.transpose(junk_ps[:, :],
                                axe[:, 176 + 128 * (i % 4):304 + 128 * (i % 4)],
                                eye72)
        js = act.tile([1, 1], f16, tag=f'junk_sb{_fill_n[0]}')
        _fill_n[0] += 1
        nc.vector.tensor_copy(js[:, :], junk_ps[0:1, 0:1])

    # ---- GIN layers (activations live feature-major between layers) ----
    x0T = tmp.tile([32, 72], f16, tag='x0T')
    tp0 = psm.tile([128, 72], f16, tag='psm')
    nc.tensor.transpose(tp0[0:32, :], x0s, eye72)
    nc.vector.tensor_copy(x0T[:, :], tp0[0:32, :])

    gi_backlog = []
    hT = x0T
    hcols = 32
    for l in range(3):
        wa = w1a if l == 0 else gw16[:, 2048:4096]
        wb = gw16[:, 0:2048] if l == 0 else gw16[:, 4096:6144]
        ba_off = 0 if l == 0 else 2 * H
        gcol = 29 if l == 0 else 37
        becol = 33 if l == 0 else 41
        bbtcol = 45 if l == 0 else 49
        nk = max(hcols // 128, 1)

        # z = h @ Wa  (halves in separate psum banks)
        z_h = [pbig.tile([72, H // 2], f32, tag='pbig', name=f'z{q}')
               for q in range(2)]
        for c in range(nk):
            cs = min(128, hcols - 128 * c)
            for q in range(2):
                rhs = (wa if l == 0 else wa[:, H * c:H * (c + 1)])[
                    :, q * (H // 2):(q + 1) * (H // 2)]
                nc.tensor.matmul(z_h[q][:, :],
                                 hT[0:cs, 72 * c:72 * (c + 1)], rhs,
                                 start=(c == 0), stop=(c == nk - 1))
        z_sb = tmp.tile([72, H], f16, tag='z_sb')
        nc.vector.tensor_copy(z_sb[:, 0:H // 2], z_h[0][:, :])
        nc.scalar.copy(z_sb[:, H // 2:], z_h[1][:, :])

        # u = Mp @ z + ba  (same half-bank split)
        u_h = [pbig.tile([72, H // 2], f32, tag='pbig', name=f'u{q}')
               for q in range(2)]
        for q in range(2):
            nc.tensor.matmul(u_h[q][:, :], m72,
                             z_sb[:, q * (H // 2):(q + 1) * (H // 2)],
                             start=True, stop=False)
        for q in range(2):
            off = ba_off + q * (H // 2)
            nc.tensor.matmul(u_h[q][:, :], ones72,
                             a3[0:1, off:off + H // 2],
                             start=False, stop=True)
        pe_filler(14)

        # LN stats: bn_stats per half, aggregated; rstd via one Rsqrt
        bst = tmp.tile([72, 12], f32, tag='bst')
        nc.vector.bn_stats(bst[:, 0:6], u_h[0][:, :])
        nc.vector.bn_stats(bst[:, 6:12], u_h[1][:, :])
        mv = tmp.tile([72, 2], f32, tag='mv')
        nc.vector.bn_aggr(mv[:, :], bst[:, :])
        std = tmp.tile([72, 1], f32, tag='std')
        nc.scalar.activation(std[:, :], mv[:, 1:2], AF.Sqrt,
                             bias=eps[:, 0:1])
        rstd = tmp.tile([72, 1], f32, tag='rstd')
        nc.vector.reciprocal(rstd[:, :], std[:, :])
        mb = tmp.tile([72, 1], f32, tag='mb')  # -mean*rstd
        nc.vector.scalar_tensor_tensor(mb[:, :], mv[:, 0:1], -1.0,
                                       rstd[:, 0:1], AL.mult, AL.mult)

        # us = (u - mean) * rstd -> fp16 (DVE half / ACT half), then
        # rT chunk = relu(us.T * g + be): PE transpose + one ACT per chunk
        us = tmp.tile([72, H], f16, tag='us')
        nc.vector.tensor_scalar(us[:, 0:H // 2], u_h[0][:, :],
                                mv[:, 0:1], rstd[:, 0:1],
                                AL.subtract, AL.mult)
        nc.scalar.activation(us[:, H // 2:], u_h[1][:, :], AF.Identity,
                             bias=mb[:, 0:1], scale=rstd[:, 0:1])
        if l == 2:
            # prefetch sigmoid/tanh table now that the last Sqrt is emitted
            sc3 = act.tile([1, 1], f32, tag='sc3')
            nc.scalar.activation(sc3[:, :], sc1[:, :], AF.Sigmoid)
        # relu(g*x+be): chunks 0,1 on ACT; chunks 2,3 on DVE (2 ops each)
        # so the four applies don't serialize on one engine
        rT = tmp.tile([128, 4 * 72], f16, tag='rT')
        for c in range(4):
            tp = psm.tile([128, 72], f16, tag='psm')
            nc.tensor.transpose(tp[:, :], us[:, 128 * c:128 * (c + 1)],
                                eye72)
            dst = rT[:, 72 * c:72 * (c + 1)]
            if c < 2:
                nc.scalar.activation(dst, tp[:, :], AF.Relu,
                                     bias=f32v[:, becol + c:becol + c + 1],
                                     scale=f32v[:, gcol + c:gcol + c + 1])
            else:
                nc.vector.tensor_scalar(dst, tp[:, :],
                                        f32v[:, gcol + c:gcol + c + 1],
                                        f32v[:, becol + c:becol + c + 1],
                                        AL.mult, AL.add)
                nc.vector.tensor_scalar_max(dst, dst, 0.0)

        # vT chunks = Wb-chunk.T @ rT-chunk (feature-major; two psum banks)
        vt_ps = [pbig.tile([128, 2 * 72], f32, tag='pvt', name=f'vt{q}')
                 for q in range(2)]
        for fi in range(4):
            for fo in range(4):
                q, o = fo % 2, fo // 2
                nc.tensor.matmul(
                    vt_ps[q][:, 72 * o:72 * (o + 1)],
                    wb[:, H * fi + 128 * fo:H * fi + 128 * fo + 128],
                    rT[:, 72 * fi:72 * (fi + 1)],
                    start=(fi == 0 and fo < 2), stop=(fi == 3),
                    skip_group_check=True)
        hnT = tmp.tile([128, 4 * 72], f16, tag='hnT')
        for fo in range(4):
            q, o = fo % 2, fo // 2
            dst = hnT[:, 72 * fo:72 * (fo + 1)]
            srcp = vt_ps[q][:, 72 * o:72 * (o + 1)]
            bb = f32v[:, bbtcol + fo:bbtcol + fo + 1]
            if fo < 2:
                nc.vector.tensor_scalar_add(dst, srcp, bb[:, 0:1])
            else:
                nc.scalar.activation(dst, srcp, AF.Identity, bias=bb[:, 0:1])

        pe_filler(10)
        # pooling: free-dim reduces per (chunk, graph) + one cast
        pf = tmp.tile([128, 8], f32, tag='pf')
        for fo in range(4):
            for g in range(2):
                nc.vector.tensor_reduce(
                    pf[:, 2 * fo + g:2 * fo + g + 1],
                    hnT[:, 72 * fo + 36 * g:72 * fo + 36 * g + 36],
                    mybir.AxisListType.X, AL.add)
        nc.vector.tensor_copy(featsT[:, 8 * l:8 * l + 8], pf[:, :])

        # queue this layer's gi matmuls (flushed later, one kc at a time)
        def make_gi(kcv):
            def emit_gi():
                for j in range(12):
                    nc.tensor.matmul(
                        gi_ps[:, 2 * j:2 * j + 2],
                        wiht_chunk(kcv, j),
                        featsT[:, 2 * kcv:2 * kcv + 2],
                        start=(kcv == 0 and j == 0), stop=(kcv == 11),
                        skip_group_check=True)
            return emit_gi
        for mc in range(4):
            gi_backlog.append(make_gi(4 * l + mc))
        hT = hnT
        hcols = H

    for kc in range(12):
        gi_backlog[kc]()
    gi_backlog = []

    # ---- GRU setup ----
    gib2 = act.tile([128, 24], f32, tag='gib2')
    nc.vector.tensor_tensor(gib2[:, :], gi_ps[:, :], f32v[:, 0:24], AL.add)
    # per graph: one [4,256] fp16 bias tile (r chunks at cols 0:128, z at
    # 128:256).  casts/copies on gpsimd so they don't block step 0's chain.
    gibT8 = []
    for g in range(2 if L > 1 else 1):
        gb8 = tmp.tile([128, 8], f16, tag='gb8')
        nc.gpsimd.tensor_copy(gb8[:, :], gib2[:, g:16:2])
        tp = psm.tile([4, 256], f16, tag='psm')
        nc.tensor.transpose(tp[:, 0:128], gb8[:, 0:4], eye128)
        nc.tensor.transpose(tp[:, 128:256], gb8[:, 4:8], eye128)
        t = act.tile([4, 256], f16, tag=f'gibT8{g}')
        nc.scalar.copy(t[:, :], tp[:, :])
        gibT8.append(t)
    if L == 1:
        gibT8.append(gibT8[0])

    # ---- GRU step 0: h=0 so gr=0; gates come straight from gib2 ----
    g0 = 0 if L > 1 else 1
    rz = tmp.tile([128, 8], f32, tag='rz')
    nc.scalar.activation(rz[:, :], gib2[:, g0:16:2], AF.Sigmoid)
    nt = tmp.tile([128, 4], f32, tag='nt')
    nc.vector.tensor_tensor(nt[:, :], rz[:, 0:4], f32v[:, 24:28], AL.mult)
    nc.vector.tensor_tensor(nt[:, :], nt[:, :], gib2[:, 16 + g0::2], AL.add)
    n = tmp.tile([128, 4], f32, tag='n')
    nc.scalar.activation(n[:, :], nt[:, :], AF.Tanh)
    w = tmp.tile([128, 4], f32, tag='w')
    nc.vector.tensor_scalar(w[:, :], rz[:, 4:8], -1.0, 1.0, AL.mult, AL.add)
    hb_a = tmp.tile([128, 2], f16, tag='hb_a')
    nc.vector.tensor_tensor(hb_a[:, :], w[:, 0:2], n[:, 0:2], AL.mult)
    hb_b = tmp.tile([128, 2], f16, tag='hb_b')
    nc.vector.tensor_tensor(hb_b[:, :], w[:, 2:4], n[:, 2:4], AL.mult)

    def hcol(c):
        return (hb_a if c < 2 else hb_b)[:, c % 2:c % 2 + 1]

    # ---- GRU steps 1..L-1 ----
    # per-gate bursts with their own closers: the r-sigmoid fires ~1/3 into
    # the matvec burst, z mid-burst; only the n-chain trails the burst.
    for t in range(1, L):
        gs = 0 if t < L - 1 else 1
        gate_ps = []
        for gate in range(3):   # 0=r, 1=z, 2=n
            g_ps = psm.tile([128, 4], f32, tag='psm')
            j0 = 4 * gate
            for c in range(4):
                for j in range(j0, j0 + 4):
                    nc.tensor.matmul(
                        g_ps[:, j - j0:j - j0 + 1],
                        whht[:, 1536 * c + 128 * j:1536 * c + 128 * (j + 1)],
                        hcol(c), start=(c == 0 and j == j0), stop=False,
                        skip_group_check=True)
            closer = (gibT8[gs][0:4, 0:128] if gate == 0 else
                      gibT8[gs][0:4, 128:256] if gate == 1 else bhhnt)
            nc.tensor.matmul(g_ps[:, :], closer, eye4,
                             start=False, stop=True, skip_group_check=True)
            gate_ps.append(g_ps)
        r_ps, z_ps, n_ps = gate_ps

        rs = tmp.tile([128, 4], f32, tag='rs')
        nc.scalar.activation(rs[:, :], r_ps[:, :], AF.Sigmoid)
        zs = tmp.tile([128, 4], f32, tag='zs')
        nc.scalar.activation(zs[:, :], z_ps[:, :], AF.Sigmoid)
        nt = tmp.tile([128, 4], f32, tag='nt')
        nc.vector.tensor_tensor(nt[:, :], rs[:, :], n_ps[:, :], AL.mult)
        nc.vector.tensor_tensor(nt[:, :], nt[:, :], gib2[:, 16 + gs::2],
                                AL.add)
        n = tmp.tile([128, 4], f32, tag='n')
        nc.scalar.activation(n[:, :], nt[:, :], AF.Tanh)
        # z-side ops on gpsimd (idle), off the DVE critical chain; they
        # complete during the burst tail + tanh
        w = tmp.tile([128, 4], f32, tag='w')
        nc.gpsimd.tensor_scalar(w[:, :], zs[:, :], -1.0, 1.0,
                                AL.mult, AL.add)
        zh = tmp.tile([128, 4], f32, tag='zh')
        nc.gpsimd.tensor_tensor(zh[:, 0:2], zs[:, 0:2], hb_a[:, :], AL.mult)
        nc.gpsimd.tensor_tensor(zh[:, 2:4], zs[:, 2:4], hb_b[:, :], AL.mult)
        wn_a = tmp.tile([128, 2], f32, tag='wn_a')
        nc.vector.tensor_tensor(wn_a[:, :], w[:, 0:2], n[:, 0:2], AL.mult)
        hb_a = tmp.tile([128, 2], f16, tag='hb_a')
        nc.vector.tensor_tensor(hb_a[:, :], wn_a[:, :], zh[:, 0:2], AL.add)
        wn_b = tmp.tile([128, 2], f32, tag='wn_b')
        nc.vector.tensor_tensor(wn_b[:, :], w[:, 2:4], n[:, 2:4], AL.mult)
        hb_b = tmp.tile([128, 2], f16, tag='hb_b')
        nc.vector.tensor_tensor(hb_b[:, :], wn_b[:, :], zh[:, 2:4], AL.add)

    # ---- classifier ----
    hid_ps = psm.tile([128, 8], f32, tag='psm')
    for mc in range(8):
        for c in range(4):
            nc.tensor.matmul(
                hid_ps[:, mc:mc + 1],
                wc1[:, 1024 * c + 128 * mc:1024 * c + 128 * (mc + 1)],
                hcol(c), start=(c == 0 and mc == 0), stop=False,
                skip_group_check=True)
    nc.tensor.matmul(hid_ps[:, :], bc1t, eye8,
                     start=False, stop=True, skip_group_check=True)
    hid = tmp.tile([128, 8], f16, tag='hid_sb')
    nc.scalar.activation(hid[:, :], hid_ps[:, :], AF.Relu)
    fin_ps = psm.tile([1, 1], f32, tag='psm')
    for mc in range(8):
        nc.tensor.matmul(fin_ps[:, :], hid[:, mc:mc + 1], wc2[:, mc:mc + 1],
                         start=(mc == 0), stop=(mc == 7))
    out_sb = tmp.tile([1, 1], f32, tag='out_sb')
    nc.scalar.activation(out_sb[:, :], fin_ps[:, :], AF.Identity,
                         bias=f32v[0:1, 28:29], scale=1.0)
    nc.sync.dma_start(out_dram, out_sb[:, :])


def _build_program(L, blobs):
    from contextlib import ExitStack
    import concourse.bacc as bacc
    import concourse.tile as tile
    import concourse.mybir as mybir

    nc = bacc.Bacc("TRN2", target_bir_lowering=False, debug=False,
                   num_devices=8)
    d = {}
    for name, arr in blobs.items():
        d[name] = nc.dram_tensor(name, list(arr.shape),
                                 mybir.dt.from_np(arr.dtype),
                                 kind="ExternalInput").ap()
    out_dram = nc.dram_tensor("out", [1], mybir.dt.float32,
                              kind="ExternalOutput").ap()
    with tile.TileContext(nc) as tc:
        with ExitStack() as ctx:
            _emit(ctx, tc, d, out_dram, L)
    nc.compile()
    return nc


def _install_ntff_hook():
    """The agent image's antenv lacks axon_hooks; recreate it so
    run_bass_kernel_spmd(trace=True) can capture NTFF profiles."""
    import sys, types
    try:
        import antenv
        if 'antenv.axon_hooks' in sys.modules:
            return
        mod = types.ModuleType('antenv.axon_hooks')
        mod._hook = None

        def set_axon_ntff_profile_hook(hk):
            mod._hook = hk

        def get_axon_ntff_profile_hook():
            return mod._hook

        mod.set_axon_ntff_profile_hook = set_axon_ntff_profile_hook
        mod.get_axon_ntff_profile_hook = get_axon_ntff_profile_hook
        sys.modules['antenv.axon_hooks'] = mod
        antenv.axon_hooks = mod
        from trn_agent_boot.trn_boot import _ntff_profile_via_ctypes
        so = '/opt/axon/libaxon_pjrt.so'
        if os.path.exists(so):
            mod._hook = _ntff_profile_via_ctypes(so)
    except Exception as e:  # profiling is best-effort
        print(f"ntff hook install failed: {e}")


def kernel(**inputs):
    from concourse.bass_utils import run_bass_kernel_spmd

    blobs, L = _prep_inputs(inputs)
    if L not in _prog_cache:
        _prog_cache[L] = _build_program(L, blobs)
    nc = _prog_cache[L]

    in_maps = [dict(blobs) for _ in range(8)]
    trace = bool(int(os.environ.get('KERNEL_TRACE', '0')))
    if trace:
        _install_ntff_hook()
    res = run_bass_kernel_spmd(nc, in_maps, list(range(8)), trace=trace)
    last_run_info['exec_time_ns'] = res.exec_time_ns
    last_run_info['results'] = res
    return np.asarray(res.results[0]['out'], np.float32).reshape(1)
